# revision 16
# baseline (speedup 1.0000x reference)
"""Dense CRF mean-field inference (2 labels) on 8 Trainium2 NeuronCores.

Strategy (column-sharded, fully SBUF-resident, col-tiled matmuls):
  - N = 80*80 = 6400 pixels. Core c owns the contiguous i-block
    [c*800, (c+1)*800).
  - All three pairwise kernels are built on-device via augmented-feature
    gram matmuls + Exp activation, stored fp16 in SBUF for all 5
    mean-field iterations (no HBM streaming).
  - The Gaussian kernel (sxy=3) is banded: only a per-core window of 20
    j-tiles (|dy| <= 10 guaranteed) is built/used; window position is a
    per-core register (dynamic APs), so one SPMD program serves all cores.
  - Mean-field matmuls for the three kernels write partitions 0:4 / 32:34
    / 64:66 of shared PSUM accumulators -> distinct PE column groups ->
    the three streams run concurrently on hardware (col tiling).
  - Row-sum normalization: one AllReduce carrying both a [128,150]
    tile-layout copy (for j-side norms) and a [3,6400] flat copy (for
    i-side norms, sliced per-core via a register-offset DMA).
  - Per-iteration Q exchange: fp16 AllGather of [800, 4].
  - All device inputs are host-prepared in their exact SBUF layouts; no
    transposing DMAs on device.
"""

import sys

if "/opt/trn_rl_repo" not in sys.path:
    sys.path.insert(0, "/opt/trn_rl_repo")

import numpy as np

import concourse.bass as bass
import concourse.tile as tile
from concourse import bacc, mybir

# ----- problem constants -----
B, H, W = 2, 80, 80
N = H * W                 # 6400
P = 128
NT = N // P               # 50 j-tiles
N_CORES = 8
CHUNK = N // N_CORES      # 800
NIC = (CHUNK + P - 1) // P  # 7 i-chunks (6x128 + 32)
I_CHUNKS = [(q * P, min(P, CHUNK - q * P)) for q in range(NIC)]
WG = 20                   # Gaussian band window tiles per core

SXY_G, COMPAT_G = 3.0, 3.0
SXY_B, SRGB_B, COMPAT_B = 50.0, 5.0, 10.0
CLIP = 1e-5

F32 = mybir.dt.float32
F16 = mybir.dt.float16
I32 = mybir.dt.int32

RG = [list(range(N_CORES))]

RS_TP = 128 * 150          # region A: [128,150] tile-layout rowsums
RS_TOT = RS_TP + 3 * N     # + region B: [3,6400] flat

_RUNNER_CACHE: dict = {}


def gt0_of(c):
    return min(max((800 * c - 800) // 128, 0), NT - WG)


def build_program(iters: int):
    nc = bacc.Bacc(
        "TRN2", target_bir_lowering=False, debug=False, num_devices=N_CORES
    )

    def inp(name, shape, dt=F32):
        return nc.dram_tensor(name, list(shape), dt, kind="ExternalInput").ap()

    # static per-core / replicated inputs (device-cached across calls)
    LgBand = inp("LgBand", (3, WG * P))  # g L rows for my band window
    biasgPT = inp("biasgPT", (P, WG))  # -|f_g|^2/2, band window, [p, t]
    ident3 = inp("ident3", (66, 8), F16)  # identity at partition bases 0/32/64
    pcore = inp("pcore", (1, 4), I32)  # [gt0, 800c, 0, 0]
    # per-call inputs (host-assembled in final layouts)
    Lball = inp("Lball", (38, N))      # bilateral L rows incl. colors
    Rall = inp("Rall", (67, CHUNK))    # all R rows (b0 0:6, b1 32:38, g 64:67)
    biasbPT = inp("biasbPT", (P, 2 * NT))  # -q2b in [p, t], b0 then b1
    nUQ = inp("nUQ", (P, NIC * 4))     # -U (negU) in [p, q, c]
    Q0s = inp("Q0s", (P, NT * 4), F16)  # Q0 in [p, t, c]
    qout = nc.dram_tensor("qout", [CHUNK, 2], F32, kind="ExternalOutput").ap()

    AF = mybir.ActivationFunctionType
    OP = mybir.AluOpType
    AX = mybir.AxisListType

    with tile.TileContext(nc) as tc:
        with (
            tc.tile_pool(name="const", bufs=1) as cpool,
            tc.tile_pool(name="dram", bufs=1, space="DRAM") as dpool,
        ):
            # ---- per-core scalars ----
            pc_sb = cpool.tile([1, 4], I32)
            nc.sync.dma_start(pc_sb[:], pcore)
            gt0 = nc.values_load(
                pc_sb[0:1, 0:1], min_val=0, max_val=NT - WG,
                skip_runtime_bounds_check=True,
            )
            c800 = nc.values_load(
                pc_sb[0:1, 1:2], min_val=0, max_val=N - CHUNK,
                skip_runtime_bounds_check=True,
            )

            # ---- persistent SBUF (lives through the iterations) ----
            kg_cache = cpool.tile([P, WG, CHUNK], F16)
            kb0_cache = cpool.tile([P, NT, CHUNK], F16)
            kb1_cache = cpool.tile([P, NT, CHUNK], F16)
            negU_sb = cpool.tile([P, NIC, 4], F32)
            ident_sb = cpool.tile([66, 8], F16)
            identf_sb = cpool.tile([8, 8], F32)
            normj = cpool.tile([P, 150], F32)
            normi = cpool.tile([P, 3, NIC], F32)
            ngrep_band = cpool.tile([P, WG, 4], F16)
            nbrep = cpool.tile([P, NT, 4], F16)
            NG4f = cpool.tile([P, NIC, 4], F32)
            NB4f = cpool.tile([P, NIC, 4], F32)

            nc.sync.dma_start(
                negU_sb[:], nUQ.rearrange("p (q c) -> p q c", c=4)
            )
            nc.sync.dma_start(ident_sb[:], ident3)
            nc.vector.tensor_copy(identf_sb[:], ident_sb[0:8, :])

            # ---------- build phase ----------
            with (
                tc.tile_pool(name="bconst", bufs=1) as bcpool,
                tc.tile_pool(name="lb", bufs=4) as lbpool,
                tc.tile_pool(name="lg", bufs=4) as lgpool,
                tc.tile_pool(name="bpsum", bufs=4, space="PSUM") as bpsum,
            ):
                biasg_sb = bcpool.tile([P, WG], F32)
                biasb_sb = bcpool.tile([P, 2 * NT], F32)
                rs_sb = bcpool.tile([P, 150], F32)
                rs_g = bcpool.tile([P, WG], F32)
                dynRb = bcpool.tile([67, CHUNK], F32)

                nc.sync.dma_start(biasg_sb[:], biasgPT)
                nc.sync.dma_start(biasb_sb[:], biasbPT)
                nc.sync.dma_start(dynRb[:], Rall)
                nc.vector.memset(rs_sb[:], 0.0)

                for t in range(NT):
                    lb = lbpool.tile([38, P], F32, tag="lb", name=f"lb{t}")
                    nc.sync.dma_start(lb[:], Lball[:, t * P : (t + 1) * P])
                    for knm, r0, rn, cache, rscol in (
                        ("b0", 0, 6, kb0_cache, rs_sb[:, 50 + t : 51 + t]),
                        ("b1", 32, 38, kb1_cache, rs_sb[:, 100 + t : 101 + t]),
                    ):
                        ps = bpsum.tile(
                            [P, CHUNK], F32, tag="bps", name=f"ps{knm}{t}"
                        )
                        for c0, cn in ((0, 512), (512, CHUNK - 512)):
                            nc.tensor.matmul(
                                ps[:, c0 : c0 + cn],
                                lhsT=lb[r0:rn, :],
                                rhs=dynRb[r0:rn, c0 : c0 + cn],
                                start=True,
                                stop=True,
                            )
                        bt_off = 0 if knm == "b0" else NT
                        nc.scalar.activation(
                            cache[:, t, :],
                            ps[:],
                            AF.Exp,
                            bias=biasb_sb[:, bt_off + t : bt_off + t + 1],
                            scale=1.0,
                        )
                        nc.vector.tensor_reduce(
                            rscol, cache[:, t, :], AX.X, OP.add
                        )
                    if t < WG:
                        bt = t
                        lg = lgpool.tile([67, P], F32, tag="lg", name=f"lg{bt}")
                        nc.sync.dma_start(
                            lg[64:67, :], LgBand[:, bt * P : (bt + 1) * P]
                        )
                        ps = bpsum.tile(
                            [P, CHUNK], F32, tag="bps", name=f"psg{bt}"
                        )
                        for c0, cn in ((0, 512), (512, CHUNK - 512)):
                            nc.tensor.matmul(
                                ps[:, c0 : c0 + cn],
                                lhsT=lg[64:67, :],
                                rhs=dynRb[64:67, c0 : c0 + cn],
                                start=True,
                                stop=True,
                            )
                        nc.scalar.activation(
                            kg_cache[:, bt, :],
                            ps[:],
                            AF.Exp,
                            bias=biasg_sb[:, bt : bt + 1],
                            scale=1.0,
                        )
                        nc.vector.tensor_reduce(
                            rs_g[:, bt : bt + 1], kg_cache[:, bt, :], AX.X,
                            OP.add,
                        )

                # place my g-band partials into global tile slots
                nc.sync.dma_start(rs_sb[:, bass.ds(gt0, WG)], rs_g[:])

                # ------ rowsum AllReduce (both layouts in one payload) ----
                rs_in = dpool.tile([RS_TOT], F32, tag="rs_in")
                rs_out = dpool.tile([RS_TOT], F32, tag="rs_out")
                nc.sync.dma_start(
                    rs_in[0:RS_TP].rearrange("(p t) -> p t", p=P), rs_sb[:]
                )
                for k in range(3):
                    nc.sync.dma_start(
                        rs_in[RS_TP + k * N : RS_TP + (k + 1) * N].rearrange(
                            "(t p) -> p t", p=P
                        ),
                        rs_sb[:, 50 * k : 50 * (k + 1)],
                    )
                nc.gpsimd.collective_compute(
                    "AllReduce",
                    OP.add,
                    replica_groups=RG,
                    ins=[rs_in.opt()],
                    outs=[rs_out.opt()],
                )

                # ---------- norms ----------
                sums_tp = bcpool.tile([P, 150], F32)
                nc.sync.dma_start(
                    sums_tp[:], rs_out[0:RS_TP].rearrange("(p t) -> p t", p=P)
                )
                recip_tp = bcpool.tile([P, 150], F32)
                nc.vector.reciprocal(recip_tp[:], sums_tp[:])
                nc.scalar.activation(normj[:], recip_tp[:], AF.Sqrt)

                # j-side norm replication (fp16)
                ngw = bcpool.tile([P, WG], F32)
                nc.sync.dma_start(ngw[:], normj[:, bass.ds(gt0, WG)])
                nc.vector.tensor_copy(
                    ngrep_band[:], ngw[:].broadcast_to([P, WG, 4])
                )
                nc.vector.tensor_copy(
                    nbrep[:, :, 0:2],
                    normj[:, 50:100].broadcast_to([P, NT, 2]),
                )
                nc.vector.tensor_copy(
                    nbrep[:, :, 2:4],
                    normj[:, 100:150].broadcast_to([P, NT, 2]),
                )

                # i-side norms for my block, from the flat region
                sums_i = bcpool.tile([P, 3, NIC], F32)
                nc.vector.memset(sums_i[:], 1.0)
                flat = rs_out[RS_TP:RS_TOT].rearrange("(k w) -> k w", k=3)
                for k in range(3):
                    nc.sync.dma_start(
                        sums_i[:, k, 0:6],
                        flat[k, bass.ds(c800, 768)].rearrange(
                            "(q p) -> p q", p=P
                        ),
                    )
                    nc.sync.dma_start(
                        sums_i[0:32, k, 6:7],
                        flat[k, bass.ds(c800 + 768, 32)].rearrange(
                            "(q p) -> p q", p=32
                        ),
                    )
                recip_i = bcpool.tile([P, 3, NIC], F32)
                nc.vector.reciprocal(recip_i[:], sums_i[:])
                nc.scalar.activation(normi[:], recip_i[:], AF.Sqrt)
                nc.vector.tensor_scalar(
                    NG4f[:],
                    normi[:, 0, :].broadcast_to([P, NIC, 4]),
                    COMPAT_G,
                    None,
                    OP.mult,
                )
                nc.vector.tensor_scalar(
                    NB4f[:, :, 0:2],
                    normi[:, 1, :].broadcast_to([P, NIC, 2]),
                    COMPAT_B,
                    None,
                    OP.mult,
                )
                nc.vector.tensor_scalar(
                    NB4f[:, :, 2:4],
                    normi[:, 2, :].broadcast_to([P, NIC, 2]),
                    COMPAT_B,
                    None,
                    OP.mult,
                )

            # ---------- mean-field iterations ----------
            with (
                tc.tile_pool(name="acc", bufs=1, space="PSUM") as accpool,
                tc.tile_pool(name="tr", bufs=2, space="PSUM") as trpool,
                tc.tile_pool(name="ep", bufs=1) as eppool,
                tc.tile_pool(name="q", bufs=2) as qpool,
            ):
                for it in range(iters):
                    last = it == iters - 1

                    q_all = qpool.tile(
                        [P, NT, 4], F16, tag="q_all", name=f"q_all{it}"
                    )
                    if it == 0:
                        nc.sync.dma_start(
                            q_all[:], Q0s.rearrange("p (t c) -> p t c", c=4)
                        )
                    else:
                        nc.sync.dma_start(
                            q_all[:],
                            qsrc.rearrange("(t p) c -> p t c", p=P),
                        )
                    rhs_b = qpool.tile(
                        [P, NT, 4], F16, tag="rhs_b", name=f"rhs_b{it}"
                    )
                    rhs_g = qpool.tile(
                        [P, WG, 4], F16, tag="rhs_g", name=f"rhs_g{it}"
                    )
                    nc.vector.tensor_tensor(
                        rhs_b[:], q_all[:], nbrep[:], OP.mult
                    )
                    q_band = qpool.tile(
                        [P, WG, 4], F16, tag="q_band", name=f"q_band{it}"
                    )
                    nc.sync.dma_start(
                        q_band[:], q_all[:, bass.ds(gt0, WG), :]
                    )
                    nc.vector.tensor_tensor(
                        rhs_g[:], q_band[:], ngrep_band[:], OP.mult
                    )

                    psA = accpool.tile([P, 512], F32, tag="pA", name=f"pA{it}")
                    psB = accpool.tile(
                        [P, CHUNK - 512], F32, tag="pB", name=f"pB{it}"
                    )
                    for t in range(NT):
                        for ps, c0, cn in (
                            (psA, 0, 512),
                            (psB, 512, CHUNK - 512),
                        ):
                            st = dict(start=(t == 0), stop=(t == NT - 1))
                            nc.tensor.matmul(
                                ps[32:34, 0:cn],
                                lhsT=rhs_b[:, t, 0:2],
                                rhs=kb0_cache[:, t, c0 : c0 + cn],
                                **st,
                            )
                            nc.tensor.matmul(
                                ps[64:66, 0:cn],
                                lhsT=rhs_b[:, t, 2:4],
                                rhs=kb1_cache[:, t, c0 : c0 + cn],
                                **st,
                            )
                            if t < WG:
                                stg = dict(
                                    start=(t == 0), stop=(t == WG - 1)
                                )
                                nc.tensor.matmul(
                                    ps[0:4, 0:cn],
                                    lhsT=rhs_g[:, t, :],
                                    rhs=kg_cache[:, t, c0 : c0 + cn],
                                    **stg,
                                )

                    # epilogue: PSUM -> fp16 SBUF copies (same partition
                    # base). b1's rows hop through DRAM from base 64 to a
                    # base-32 tile (direct partition-shift paths and
                    # base-64 flips are broken on this toolchain). Flips:
                    # g via transpose-mode @0, b0/b1 via regular matmul
                    # against identity @(32, 0) - all probed-good.
                    sgg = eppool.tile([4, CHUNK], F32, tag="sgg", name=f"sg{it}")
                    sb0 = eppool.tile([34, CHUNK], F16, tag="sb0", name=f"s0{it}")
                    sb1t = eppool.tile([66, CHUNK], F16, tag="sb1t", name=f"s1{it}")
                    sb1m = eppool.tile([34, CHUNK], F16, tag="sb1m", name=f"sm{it}")
                    for ps, c0, cn in ((psA, 0, 512), (psB, 512, CHUNK - 512)):
                        nc.vector.tensor_copy(
                            sgg[:, c0 : c0 + cn], ps[0:4, 0:cn]
                        )
                        nc.vector.tensor_copy(
                            sb0[32:34, c0 : c0 + cn], ps[32:34, 0:cn]
                        )
                        nc.vector.tensor_copy(
                            sb1t[64:66, c0 : c0 + cn], ps[64:66, 0:cn]
                        )
                    b1hop = dpool.tile(
                        [2, CHUNK], F16, tag="b1hop", name=f"b1hop{it}"
                    )
                    nc.sync.dma_start(b1hop[:], sb1t[64:66, :])
                    nc.sync.dma_start(sb1m[32:34, :], b1hop[:])

                    trP = trpool.tile(
                        [P, NIC, 8], F32, tag="trP", name=f"trP{it}"
                    )
                    nc.vector.memset(trP[:], 0.0)
                    for q, (i0, iw) in enumerate(I_CHUNKS):
                        nc.tensor.transpose(
                            trP[0:iw, q, 0:4],
                            sgg[:, i0 : i0 + iw],
                            identf_sb[0:4, 0:4],
                        )
                        nc.tensor.matmul(
                            trP[0:iw, q, 4:6],
                            lhsT=sb0[32:34, i0 : i0 + iw],
                            rhs=ident_sb[32:34, 0:2],
                            start=True,
                            stop=True,
                        )
                        nc.tensor.matmul(
                            trP[0:iw, q, 6:8],
                            lhsT=sb1m[32:34, i0 : i0 + iw],
                            rhs=ident_sb[32:34, 0:2],
                            start=True,
                            stop=True,
                        )
                    trPs = eppool.tile(
                        [P, NIC, 8], F32, tag="trPs", name=f"trPs{it}"
                    )
                    nc.vector.tensor_copy(trPs[:], trP[:])
                    ep1 = eppool.tile([P, NIC, 4], F32, tag="ep1", name=f"e1{it}")
                    ep2 = eppool.tile([P, NIC, 4], F32, tag="ep2", name=f"e2{it}")
                    nc.vector.tensor_tensor(
                        ep1[:], trPs[:, :, 0:4], NG4f[:], OP.mult
                    )
                    nc.vector.tensor_tensor(
                        ep2[:], trPs[:, :, 4:8], NB4f[:], OP.mult
                    )
                    nc.vector.tensor_tensor(ep1[:], ep1[:], ep2[:], OP.add)
                    nc.vector.tensor_tensor(ep1[:], ep1[:], negU_sb[:], OP.add)
                    d = eppool.tile([P, NIC, 2], F32, tag="d", name=f"d{it}")
                    nc.vector.tensor_tensor(
                        d[:],
                        ep1[:, :, 0:4:2],
                        ep1[:, :, 1:4:2],
                        OP.subtract,
                    )

                    if last:
                        qs32 = eppool.tile(
                            [P, NIC, 2], F32, tag="qs32", name=f"qs32{it}"
                        )
                        nc.scalar.activation(qs32[:], d[:], AF.Sigmoid)
                        nc.sync.dma_start(
                            qout[0 : 6 * P, :].rearrange(
                                "(q p) c -> p q c", p=P
                            ),
                            qs32[:, 0:6, :],
                        )
                        nc.sync.dma_start(
                            qout[6 * P : CHUNK, :], qs32[0:32, 6, :]
                        )
                    else:
                        qstage = qpool.tile(
                            [P, NIC, 4], F16, tag="qstage", name=f"qst{it}"
                        )
                        nc.scalar.activation(
                            qstage[:, :, 0:4:2], d[:], AF.Sigmoid
                        )
                        nc.scalar.activation(
                            qstage[:, :, 1:4:2], d[:], AF.Sigmoid, scale=-1.0
                        )
                        qag_in = dpool.tile(
                            [CHUNK, 4], F16, tag=f"qin{it}", name=f"qin{it}"
                        )
                        qag_out = dpool.tile(
                            [N, 4], F16, tag=f"qout{it}", name=f"qo{it}"
                        )
                        nc.sync.dma_start(
                            qag_in[0 : 6 * P, :].rearrange(
                                "(q p) c -> p q c", p=P
                            ),
                            qstage[:, 0:6, :],
                        )
                        nc.sync.dma_start(
                            qag_in[6 * P : CHUNK, :], qstage[0:32, 6, :]
                        )
                        nc.gpsimd.collective_compute(
                            "AllGather",
                            OP.bypass,
                            replica_groups=RG,
                            ins=[qag_in.opt()],
                            outs=[qag_out.opt()],
                        )
                        qsrc = qag_out

    nc.compile()
    return nc


# ---------------- host-side data ----------------

def _pos_features():
    yy, xx = np.mgrid[0:H, 0:W]
    pos = np.stack([xx.ravel(), yy.ravel()], 1).astype(np.float64)  # [N,2]
    return pos


def static_inputs():
    pos = _pos_features()
    fg = pos / SXY_G
    q2g = 0.5 * (fg * fg).sum(1)
    fb = pos / SXY_B

    f32 = lambda a: np.ascontiguousarray(a, dtype=np.float32)

    ident3 = np.zeros((66, 8), np.float16)
    ident3[0:8, 0:8] = np.eye(8)
    ident3[32:40, 0:8] = np.eye(8)[0:8]
    ident3[64:66, 0:2] = np.eye(2)

    shared = {"ident3": ident3}

    maps = []
    for c in range(N_CORES):
        g0 = gt0_of(c)
        jw = slice(g0 * P, (g0 + WG) * P)
        m = dict(shared)
        m["LgBand"] = f32(
            np.concatenate([fg[jw].T, np.ones((1, WG * P))], 0)
        )
        m["biasgPT"] = f32(
            -q2g[jw].reshape(WG, P).T
        )
        m["pcore"] = np.array(
            [[g0, c * CHUNK, 0, 0]], dtype=np.int32
        )
        maps.append(m)
    return maps


def fresh_inputs(img: np.ndarray, pred: np.ndarray):
    """Per-call inputs: colors, -q2b, negU, Q0 (host layouts)."""
    colors = img.reshape(B, 3, N).transpose(0, 2, 1).astype(np.float64) * 255.0
    cb = colors / SRGB_B                                # [B,N,3]
    pos = _pos_features()
    fb = pos / SXY_B
    q2b = np.stack(
        [0.5 * ((fb * fb).sum(1) + (cb[b] * cb[b]).sum(1)) for b in range(B)]
    )  # [B, N]

    p = pred.reshape(B, N).astype(np.float64)
    probs = np.clip(np.stack([p, 1.0 - p], -1), CLIP, 1.0)  # [B,N,2]
    negU = np.log(probs)
    Q0 = probs / probs.sum(-1, keepdims=True)

    f32 = lambda a: np.ascontiguousarray(a, dtype=np.float32)

    fg = pos / SXY_G
    q2g = 0.5 * (fg * fg).sum(1)
    Lball = np.zeros((38, N))
    Lball[0:2] = fb.T
    Lball[2:5] = cb[0].T
    Lball[5] = 1.0
    Lball[32:34] = fb.T
    Lball[34:37] = cb[1].T
    Lball[37] = 1.0
    biasbPT = f32(
        np.concatenate(
            [-q2b[0].reshape(NT, P).T, -q2b[1].reshape(NT, P).T], 1
        )
    )  # [128, 100]
    # Q0 in stationary layout [p, t, c], c = 2*img + label
    Q0s = np.zeros((P, NT, 4), np.float16)
    q0r = Q0.transpose(1, 0, 2).reshape(N, 4)  # [N, (img, label)]
    Q0s[:, :, :] = q0r.reshape(NT, P, 4).transpose(1, 0, 2)
    Q0s = np.ascontiguousarray(Q0s.reshape(P, NT * 4))

    rep = {"Lball": f32(Lball), "biasbPT": biasbPT, "Q0s": Q0s}
    per_core = []
    for c in range(N_CORES):
        sl = slice(c * CHUNK, (c + 1) * CHUNK)
        m = dict(rep)
        Rall = np.zeros((67, CHUNK))
        Rall[0:2] = fb[sl].T
        Rall[2:5] = cb[0][sl].T
        Rall[5] = -q2b[0, sl]
        Rall[32:34] = fb[sl].T
        Rall[34:37] = cb[1][sl].T
        Rall[37] = -q2b[1, sl]
        Rall[64:66] = fg[sl].T
        Rall[66] = -q2g[sl]
        m["Rall"] = f32(Rall)
        nu = np.zeros((P, NIC, 4), np.float64)
        nUr = negU.transpose(1, 0, 2).reshape(N, 4)[sl]  # [800, 4]
        nu[:, 0:6, :] = nUr[0 : 6 * P].reshape(6, P, 4).transpose(1, 0, 2)
        nu[0:32, 6, :] = nUr[6 * P : CHUNK]
        m["nUQ"] = f32(nu.reshape(P, NIC * 4))
        per_core.append(m)
    return per_core, Q0


# ---------------- PJRT runner (cached across calls) ----------------

class _Runner:
    def __init__(self, iters: int):
        import jax
        from jax.sharding import Mesh, PartitionSpec, NamedSharding
        from jax.experimental.shard_map import shard_map
        from concourse import bass2jax

        self.jax = jax
        nc = build_program(iters)
        bass2jax.install_neuronx_cc_hook()

        partition_name = (
            nc.partition_id_tensor.name if nc.partition_id_tensor else None
        )
        in_names, out_names, out_avals = [], [], []
        zero_outs = []
        for alloc in nc.m.functions[0].allocations:
            if not isinstance(alloc, mybir.MemoryLocationSet):
                continue
            name = alloc.memorylocations[0].name
            if alloc.kind == "ExternalInput":
                if name != partition_name:
                    in_names.append(name)
            elif alloc.kind == "ExternalOutput":
                shape = tuple(alloc.tensor_shape)
                dtype = mybir.dt.np(alloc.dtype)
                out_names.append(name)
                out_avals.append(jax.core.ShapedArray(shape, dtype))
                zero_outs.append(np.zeros(shape, dtype))
        self.in_names = in_names
        self.out_names = out_names
        self.out_avals = out_avals
        all_in_names = list(in_names) + list(out_names)
        if partition_name is not None:
            all_in_names.append(partition_name)

        def _body(*args):
            operands = list(args)
            if partition_name is not None:
                operands.append(bass2jax.partition_id_tensor())
            outs = bass2jax._bass_exec_p.bind(
                *operands,
                out_avals=tuple(out_avals),
                in_names=tuple(all_in_names),
                out_names=tuple(out_names),
                lowering_input_output_aliases=(),
                sim_require_finite=True,
                sim_require_nnan=True,
                nc=nc,
            )
            return tuple(outs)

        devices = jax.devices()[:N_CORES]
        mesh = Mesh(np.asarray(devices), ("core",))
        n_in = len(in_names) + len(zero_outs)
        sharded = jax.jit(
            shard_map(
                _body,
                mesh=mesh,
                in_specs=(PartitionSpec("core"),) * n_in,
                out_specs=(PartitionSpec("core"),) * len(out_names),
                check_rep=False,
            ),
            keep_unused=True,
        )
        self.sharded = sharded
        self._body = _body
        self.mesh = mesh
        sh = NamedSharding(mesh, PartitionSpec("core"))
        self.sh = sh

        smaps = static_inputs()
        self.static = {
            nm: jax.device_put(
                np.concatenate([smaps[c][nm] for c in range(N_CORES)], 0), sh
            )
            for nm in smaps[0]
        }
        self.zeros = [
            jax.device_put(
                np.zeros((N_CORES * z.shape[0], *z.shape[1:]), z.dtype), sh
            )
            for z in zero_outs
        ]
        jax.block_until_ready(list(self.static.values()))
        jax.block_until_ready(self.zeros)
        self._fresh_cache = {}

    def args_for(self, fresh_per_core, device_put_fresh=False):
        args = []
        for nm in self.in_names:
            if nm in fresh_per_core[0]:
                a = np.concatenate([m[nm] for m in fresh_per_core], 0)
                if device_put_fresh:
                    a = self.jax.device_put(a, self.sh)
                args.append(a)
            else:
                args.append(self.static[nm])
        return args

    def __call__(self, fresh_per_core):
        outs = self.sharded(*self.args_for(fresh_per_core), *self.zeros)
        q = np.asarray(outs[self.out_names.index("qout")])
        return q.reshape(N_CORES * CHUNK, 2)


def get_runner(iters: int) -> "_Runner":
    if iters not in _RUNNER_CACHE:
        _RUNNER_CACHE[iters] = _Runner(iters)
    return _RUNNER_CACHE[iters]


def kernel(img, pred, iters):
    img = np.asarray(img, dtype=np.float32)
    pred = np.asarray(pred, dtype=np.float32)
    iters = int(np.asarray(iters))

    fresh, Q0 = fresh_inputs(img, pred)
    if iters <= 0:
        return np.ascontiguousarray(
            Q0[..., 0].astype(np.float32).reshape(B, 1, H, W)
        )

    runner = get_runner(iters)
    prob0 = runner(fresh)  # [N, 2], columns = image index
    out = np.stack(
        [prob0[:, 0].reshape(1, H, W), prob0[:, 1].reshape(1, H, W)], axis=0
    ).astype(np.float32)
    return out
